# revision 3
# baseline (speedup 1.0000x reference)
"""Multi-head attention (B=4, S=2048, E=1024, H=16, D=64) on 8 trn2 cores.

Sharding: core c -> (batch b = c//2, head-group hg = c%2 of 8 heads).
Each core computes its 8 heads' attention for its batch plus the partial
output projection (its 512 rows of w_proj); the host sums the two partials
per batch and adds the folded bias (b_proj + b_v @ w_proj).

v3 schedule (final): QKV stripes spread evenly through all attention
blocks (two per block) so the PE has filler work wherever the Act
engine is the per-block limiter.

v2 restructure vs baseline:
  - x is DMA'd ONCE as bf16 [128, 8, 2048] and kept in SBUF (the baseline
    re-streamed x from DRAM five times, leaving phase 1 DMA-bound).
  - all weights are pre-converted to bf16 on host; every matmul runs in
    bf16 (QKV error stays ~1e-3, far under the 2e-2 gate).
  - emission interleaves: QKV(pair 0) -> V -> [attention(pair j) with one
    QKV stripe of pair j+1 woven between q-tiles] -> projection, so the
    scalar engine (exp) gets work early and the PE never drains.
  - normalization reads PV psum once into SBUF (freeing the accumulator
    bank fast) and the broadcast matmuls are deferred into the next
    block's PE stream so the PE doesn't stall on the DVE recip chain.
"""

import ml_dtypes
import numpy as np

S = 2048
E = 1024
NCORES = 8

_PROGRAM = None
TRACE = False
LAST_RESULT = None


def _build_body(tc, t, o, s_len):
    import concourse.bass as bass  # noqa: F401
    from concourse import mybir

    nc = tc.nc
    f32 = mybir.dt.float32
    f32r = mybir.dt.float32r
    bf16 = mybir.dt.bfloat16
    AF = mybir.ActivationFunctionType
    ALU = mybir.AluOpType

    ST = s_len // 512   # number of 512-wide s/q tiles
    KC = s_len // 128   # number of 128-row k chunks

    def r(ap):
        return ap.bitcast(f32r)

    with tc.tile_pool(name="const", bufs=1) as constp, \
         tc.tile_pool(name="big", bufs=1) as bigp:
        X = bigp.tile([128, 8, s_len], bf16, name="X")
        QT = bigp.tile([128, 4, s_len], bf16, name="QT")
        KT = bigp.tile([128, 4, s_len], bf16, name="KT")
        V = bigp.tile([128, KC, 8, 65], bf16, name="V")
        WQ = bigp.tile([128, 8, 512], bf16, name="WQ")
        WK = bigp.tile([128, 8, 512], bf16, name="WK")
        WV = bigp.tile([128, 8, 512], bf16, name="WV")
        WP = bigp.tile([128, 4, 1024], bf16, name="WP")
        HT = bigp.tile([128, ST, 4, 512], bf16, name="HT")
        BQ = constp.tile([128, 4], f32, name="BQ")
        BK = constp.tile([128, 4], f32, name="BK")

        nc.sync.dma_start(BQ, t["bq"])
        nc.sync.dma_start(BK, t["bk"])
        nc.sync.dma_start(WQ, t["wq"].rearrange("(c p) d -> p c d", p=128))
        nc.sync.dma_start(WK, t["wk"].rearrange("(c p) d -> p c d", p=128))
        nc.sync.dma_start(WV, t["wv"].rearrange("(c p) d -> p c d", p=128))
        nc.sync.dma_start(WP, t["wp"].rearrange("(c p) e -> p c e", p=128))
        nc.sync.dma_start(
            V[:, :, :, 64],
            t["onesb"][:, 0:KC * 8].rearrange("p (c h) -> p c h", h=8),
        )
        # chunked X load so the first K-stripe matmul (c=0) starts after
        # 1/8th of the transfer
        xs = t["xT"].rearrange("(c p) s -> p c s", p=128)
        for c in range(8):
            nc.sync.dma_start(X[:, c, :], xs[:, c, :])

        with tc.tile_pool(name="pp", bufs=2, space="PSUM") as ppool, \
             tc.tile_pool(name="at", bufs=6) as atp, \
             tc.tile_pool(name="iv", bufs=2) as ivp, \
             tc.tile_pool(name="ob", bufs=2) as obp, \
             tc.tile_pool(name="sc", bufs=2, space="PSUM") as scp, \
             tc.tile_pool(name="ot", bufs=2, space="PSUM") as otp:

            def emit_q_stripe(j, st):
                ss = slice(st * 512, (st + 1) * 512)
                qp = ppool.tile([128, 512], f32, name="pp")
                for c in range(8):
                    nc.tensor.matmul(
                        qp, WQ[:, c, j * 128:(j + 1) * 128], X[:, c, ss],
                        start=(c == 0), stop=(c == 7),
                    )
                # QT = 0.125 * (x@wq) + 0.125*bq   (bq pre-scaled on host)
                nc.vector.tensor_scalar(
                    QT[:, j, ss], qp, 0.125, BQ[:, j:j + 1], ALU.mult, ALU.add,
                )

            def emit_k_stripe(j, st):
                ss = slice(st * 512, (st + 1) * 512)
                kp = ppool.tile([128, 512], f32, name="pp")
                for c in range(8):
                    nc.tensor.matmul(
                        kp, WK[:, c, j * 128:(j + 1) * 128], X[:, c, ss],
                        start=(c == 0), stop=(c == 7),
                    )
                nc.vector.tensor_scalar(
                    KT[:, j, ss], kp, BK[:, j:j + 1], None, ALU.add,
                )

            def emit_v_chunk(kc):
                vp = ppool.tile([128, 512], f32, name="pp")
                s0 = kc * 128
                for c in range(8):
                    nc.tensor.matmul(
                        vp, X[:, c, s0:s0 + 128], WV[:, c, :],
                        start=(c == 0), stop=(c == 7),
                    )
                nc.vector.tensor_copy(
                    V[:, kc, :, 0:64],
                    vp.rearrange("p (h d) -> p h d", d=64),
                )

            def attn(j, qt, fillers=()):
                # fillers: closures (QKV stripes / V chunks / proj quarters)
                # woven one-per-tt into the PE stream starting at tt=2, so
                # the PE chews prep work while the Act engine exps
                fillers = list(fillers)
                assert len(fillers) <= KC - 2
                qs_ = slice(qt * 512, (qt + 1) * 512)
                outA = otp.tile([128, 512], f32, name="ot")
                outB = otp.tile([128, 512], f32, name="ot")
                for tt in range(KC):
                    sc = scp.tile([128, 1024], f32, name="sc")
                    ks = slice(tt * 128, (tt + 1) * 128)
                    nc.tensor.matmul(
                        sc[:, 0:512], KT[0:64, j, ks], QT[0:64, j, qs_],
                        start=True, stop=True,
                    )
                    nc.tensor.matmul(
                        sc[:, 512:1024], KT[64:128, j, ks], QT[64:128, j, qs_],
                        start=True, stop=True,
                    )
                    at = atp.tile([128, 1024], bf16, name="at")
                    nc.scalar.activation(at, sc, AF.Exp)
                    if tt >= 2 and fillers:
                        fillers.pop(0)()
                    nc.tensor.matmul(
                        outA[0:65, :], V[:, tt, 2 * j, :], at[:, 0:512],
                        start=(tt == 0), stop=(tt == KC - 1),
                    )
                    nc.tensor.matmul(
                        outB[0:65, :], V[:, tt, 2 * j + 1, :], at[:, 512:1024],
                        start=(tt == 0), stop=(tt == KC - 1),
                    )
                for f in fillers:
                    f()
                # normalization is PE-free: DVE copy+recip frees the psum
                # accumulators, Pool broadcasts 1/den across partitions,
                # DVE multiplies into HT
                UA = ivp.tile([65, 512], f32, name="UA")
                UB = ivp.tile([65, 512], f32, name="UB")
                dnA = ivp.tile([1, 512], f32, name="dnA")
                dnB = ivp.tile([1, 512], f32, name="dnB")
                ivA = ivp.tile([1, 512], f32, name="ivA")
                ivB = ivp.tile([1, 512], f32, name="ivB")
                bcA = ivp.tile([64, 512], f32, name="bcA")
                bcB = ivp.tile([64, 512], f32, name="bcB")
                nc.vector.tensor_copy(UA, outA[0:65, :])
                nc.vector.tensor_copy(UB, outB[0:65, :])
                # partition_broadcast only reads partition 0 (and DMA cannot
                # read psum): hop the denominator row down via SBUF->SBUF DMA
                nc.sync.dma_start(dnA, UA[64:65, :])
                nc.sync.dma_start(dnB, UB[64:65, :])
                with nc.allow_low_precision(reason="softmax denom"):
                    nc.vector.reciprocal(ivA, dnA)
                    nc.vector.reciprocal(ivB, dnB)
                nc.gpsimd.partition_broadcast(bcA, ivA)
                nc.gpsimd.partition_broadcast(bcB, ivB)
                # head A lives at HT partitions 0:64 directly
                nc.vector.tensor_mul(HT[0:64, qt, j, :], UA[0:64, :], bcA)
                # head B: compute at base 0, DMA-move to partitions 64:128
                stg = ivp.tile([64, 512], bf16, name="stg")
                nc.vector.tensor_mul(stg, UB[0:64, :], bcB)
                nc.sync.dma_start(HT[64:128, qt, j, :], stg)

            def proj_quarter(qt, q4):
                ob = obp.tile([128, 1024], f32, name="ob")
                rs = slice(q4 * 128, (q4 + 1) * 128)
                for half in range(2):
                    pj = ppool.tile([128, 512], f32, name="pp")
                    for c in range(4):
                        nc.tensor.matmul(
                            pj, HT[:, qt, c, rs],
                            WP[:, c, half * 512:(half + 1) * 512],
                            start=(c == 0), stop=(c == 3),
                        )
                    nc.vector.tensor_copy(
                        ob[:, half * 512:(half + 1) * 512], pj
                    )
                r0 = qt * 512 + q4 * 128
                nc.sync.dma_start(o[r0:r0 + 128, :], ob)

            # ---------- emission schedule ----------
            def F(fn, *a):
                return lambda: fn(*a)

            Q, K, VC, PQ = emit_q_stripe, emit_k_stripe, emit_v_chunk, proj_quarter
            for st in range(ST):
                emit_k_stripe(0, st)
            emit_q_stripe(0, 0)
            emit_q_stripe(0, 1)
            for kc in range(4):
                emit_v_chunk(kc)
            attn(0, 0, [F(VC, kc) for kc in range(4, KC)])
            attn(0, 1, [F(Q, 0, 2), F(Q, 0, 3), F(K, 1, 0)])
            attn(0, 2, [F(K, 1, 1), F(K, 1, 2), F(Q, 1, 0)])
            attn(0, 3, [F(K, 1, 3), F(Q, 1, 1), F(Q, 1, 2)])
            attn(1, 0, [F(Q, 1, 3), F(K, 2, 0)])
            attn(1, 1, [F(K, 2, 1), F(Q, 2, 0)])
            attn(1, 2, [F(K, 2, 2), F(Q, 2, 1)])
            attn(1, 3, [F(K, 2, 3), F(Q, 2, 2)])
            attn(2, 0, [F(Q, 2, 3), F(K, 3, 0)])
            attn(2, 1, [F(K, 3, 1), F(Q, 3, 0)])
            attn(2, 2, [F(K, 3, 2), F(Q, 3, 1)])
            attn(2, 3, [F(K, 3, 3), F(Q, 3, 2)])
            attn(3, 0, [F(Q, 3, 3)])
            attn(3, 1, [F(PQ, 0, q4) for q4 in range(4)])
            attn(3, 2, [F(PQ, 1, q4) for q4 in range(4)])
            attn(3, 3, [F(PQ, 2, q4) for q4 in range(4)])
            for q4 in range(4):
                proj_quarter(3, q4)


def _build_program(s_len=S, repeat=1, timing=False):
    import concourse.bacc as bacc
    import concourse.tile as tile
    from concourse import mybir

    f32 = mybir.dt.float32
    bf16 = mybir.dt.bfloat16
    # timing=True: all real tensors Internal (no host transfer; garbage data
    # has identical cycle counts) + a tiny dummy output, so wall-clock is
    # dispatch + device time only
    kin = "Internal" if timing else "ExternalInput"
    kout = "Internal" if timing else "ExternalOutput"
    nc = bacc.Bacc(
        "TRN2", target_bir_lowering=False, debug=False, num_devices=NCORES
    )
    t = {
        "xT": nc.dram_tensor("xT", [E, s_len], bf16, kind=kin).ap(),
        "wq": nc.dram_tensor("wq", [E, 512], bf16, kind=kin).ap(),
        "wk": nc.dram_tensor("wk", [E, 512], bf16, kind=kin).ap(),
        "wv": nc.dram_tensor("wv", [E, 512], bf16, kind=kin).ap(),
        "wp": nc.dram_tensor("wp", [512, E], bf16, kind=kin).ap(),
        "bq": nc.dram_tensor("bq", [128, 4], f32, kind=kin).ap(),
        "bk": nc.dram_tensor("bk", [128, 4], f32, kind=kin).ap(),
        "ones": nc.dram_tensor("ones", [128, 128], f32, kind=kin).ap(),
        "onesb": nc.dram_tensor(
            "onesb", [128, 128], bf16, kind=kin
        ).ap(),
    }
    o = nc.dram_tensor("o", [s_len, E], f32, kind=kout).ap()
    dummy = (
        nc.dram_tensor("tdum", [1, 4], f32, kind="ExternalOutput").ap()
        if timing else None
    )
    with tile.TileContext(nc) as tc:
        if repeat > 1:
            with tc.For_i(0, repeat, 1):
                _build_body(tc, t, o, s_len)
        else:
            _build_body(tc, t, o, s_len)
        if dummy is not None:
            with tc.tile_pool(name="dum", bufs=1) as dp:
                dt_ = dp.tile([1, 4], f32, name="dum")
                nc.vector.memset(dt_, 0.0)
                nc.sync.dma_start(dummy, dt_)
    nc.compile()
    return nc


def _get_program():
    global _PROGRAM
    if _PROGRAM is None:
        _PROGRAM = _build_program()
    return _PROGRAM


def _shard_inputs(x, w_qkv, b_qkv, w_proj):
    bf16 = ml_dtypes.bfloat16
    wq_f, wk_f, wv_f = w_qkv[:, :E], w_qkv[:, E:2 * E], w_qkv[:, 2 * E:]
    bq_f, bk_f = b_qkv[:E], b_qkv[E:2 * E]
    in_maps = []
    for c in range(NCORES):
        b, hg = divmod(c, 2)
        sl = slice(hg * 512, (hg + 1) * 512)
        in_maps.append({
            "xT": np.ascontiguousarray(x[b].T).astype(bf16),
            "wq": np.ascontiguousarray(wq_f[:, sl]).astype(bf16),
            "wk": np.ascontiguousarray(wk_f[:, sl]).astype(bf16),
            "wv": np.ascontiguousarray(wv_f[:, sl]).astype(bf16),
            "wp": np.ascontiguousarray(w_proj[sl, :]).astype(bf16),
            "bq": np.ascontiguousarray((bq_f[sl] * 0.125).reshape(4, 128).T),
            "bk": np.ascontiguousarray(bk_f[sl].reshape(4, 128).T),
            "ones": np.ones((128, 128), np.float32),
            "onesb": np.ones((128, 128), bf16),
        })
    return in_maps


def kernel(x, w_qkv, b_qkv, w_proj, b_proj):
    global LAST_RESULT
    from concourse.bass_utils import run_bass_kernel_spmd

    x = np.asarray(x, dtype=np.float32)
    w_qkv = np.asarray(w_qkv, dtype=np.float32)
    b_qkv = np.asarray(b_qkv, dtype=np.float32)
    w_proj = np.asarray(w_proj, dtype=np.float32)
    b_proj = np.asarray(b_proj, dtype=np.float32)

    nc = _get_program()
    in_maps = _shard_inputs(x, w_qkv, b_qkv, w_proj)
    res = run_bass_kernel_spmd(nc, in_maps, list(range(NCORES)), trace=TRACE)
    LAST_RESULT = res

    bv_f = b_qkv[2 * E:]
    b_eff = (b_proj + bv_f @ w_proj).astype(np.float32)
    out = np.empty((4, S, E), dtype=np.float32)
    for b in range(4):
        out[b] = res.results[2 * b]["o"] + res.results[2 * b + 1]["o"] + b_eff
    return out


# revision 4
# speedup vs baseline: 1.0152x; 1.0152x over previous
"""Multi-head attention (B=4, S=2048, E=1024, H=16, D=64) on 8 trn2 cores.

Sharding: core c -> (batch b = c//2, head-group hg = c%2 of 8 heads).
Each core computes its 8 heads' attention for its batch plus the partial
output projection (its 512 rows of w_proj); the host sums the two partials
per batch and adds the folded bias (b_proj + b_v @ w_proj).

v2 restructure vs baseline:
  - x is DMA'd ONCE as bf16 [128, 8, 2048] and kept in SBUF (the baseline
    re-streamed x from DRAM five times, leaving phase 1 DMA-bound).
  - all weights are pre-converted to bf16 on host; every matmul runs in
    bf16 (QKV error stays ~1e-3, far under the 2e-2 gate).
  - emission interleaves: QKV(pair 0) -> V -> [attention(pair j) with one
    QKV stripe of pair j+1 woven between q-tiles] -> projection, so the
    scalar engine (exp) gets work early and the PE never drains.
  - normalization reads PV psum once into SBUF (freeing the accumulator
    bank fast) and the broadcast matmuls are deferred into the next
    block's PE stream so the PE doesn't stall on the DVE recip chain.
"""

import ml_dtypes
import numpy as np

S = 2048
E = 1024
NCORES = 8

_PROGRAM = None
TRACE = False
LAST_RESULT = None


def _build_body(tc, t, o, s_len):
    import concourse.bass as bass  # noqa: F401
    from concourse import mybir

    nc = tc.nc
    f32 = mybir.dt.float32
    f32r = mybir.dt.float32r
    bf16 = mybir.dt.bfloat16
    AF = mybir.ActivationFunctionType
    ALU = mybir.AluOpType

    ST = s_len // 512   # number of 512-wide s/q tiles
    KC = s_len // 128   # number of 128-row k chunks

    def r(ap):
        return ap.bitcast(f32r)

    with tc.tile_pool(name="const", bufs=1) as constp, \
         tc.tile_pool(name="big", bufs=1) as bigp:
        X = bigp.tile([128, 8, s_len], bf16, name="X")
        QT = bigp.tile([128, 4, s_len], bf16, name="QT")
        KT = bigp.tile([128, 4, s_len], bf16, name="KT")
        V = bigp.tile([128, KC, 8, 65], bf16, name="V")
        WQ = bigp.tile([128, 8, 512], bf16, name="WQ")
        WK = bigp.tile([128, 8, 512], bf16, name="WK")
        WV = bigp.tile([128, 8, 512], bf16, name="WV")
        WP = bigp.tile([128, 4, 1024], bf16, name="WP")
        HT = bigp.tile([128, ST, 4, 512], bf16, name="HT")
        BQ = constp.tile([128, 4], f32, name="BQ")
        BK = constp.tile([128, 4], f32, name="BK")

        nc.sync.dma_start(BQ, t["bq"])
        nc.sync.dma_start(BK, t["bk"])
        nc.sync.dma_start(WQ, t["wq"].rearrange("(c p) d -> p c d", p=128))
        nc.sync.dma_start(WK, t["wk"].rearrange("(c p) d -> p c d", p=128))
        # ones column for the PV denominator trick: engine-tracked memset
        # (a DMA here has a racy dependency edge against the first PV reads)
        nc.vector.memset(V[:, :, :, 64], 1.0)
        # X ahead of WV/WP: the first K/Q stripes need all of X, while the
        # V matmuls start ~26us in and the projection ~300us in
        xs = t["xT"].rearrange("(c p) s -> p c s", p=128)
        for c in range(8):
            nc.sync.dma_start(X[:, c, :], xs[:, c, :])
        nc.sync.dma_start(WV, t["wv"].rearrange("(c p) d -> p c d", p=128))
        nc.sync.dma_start(WP, t["wp"].rearrange("(c p) e -> p c e", p=128))

        with tc.tile_pool(name="pp", bufs=2, space="PSUM") as ppool, \
             tc.tile_pool(name="at", bufs=6) as atp, \
             tc.tile_pool(name="iv", bufs=2) as ivp, \
             tc.tile_pool(name="ob", bufs=2) as obp, \
             tc.tile_pool(name="sc", bufs=2, space="PSUM") as scp, \
             tc.tile_pool(name="ot", bufs=2, space="PSUM") as otp:

            def emit_q_stripe(j, st):
                ss = slice(st * 512, (st + 1) * 512)
                qp = ppool.tile([128, 512], f32, name="pp")
                for c in range(8):
                    nc.tensor.matmul(
                        qp, WQ[:, c, j * 128:(j + 1) * 128], X[:, c, ss],
                        start=(c == 0), stop=(c == 7),
                    )
                # QT = 0.125 * (x@wq) + 0.125*bq   (bq pre-scaled on host)
                nc.vector.tensor_scalar(
                    QT[:, j, ss], qp, 0.125, BQ[:, j:j + 1], ALU.mult, ALU.add,
                )

            def emit_k_stripe(j, st):
                ss = slice(st * 512, (st + 1) * 512)
                kp = ppool.tile([128, 512], f32, name="pp")
                for c in range(8):
                    nc.tensor.matmul(
                        kp, WK[:, c, j * 128:(j + 1) * 128], X[:, c, ss],
                        start=(c == 0), stop=(c == 7),
                    )
                nc.vector.tensor_scalar(
                    KT[:, j, ss], kp, BK[:, j:j + 1], None, ALU.add,
                )

            def emit_v_chunk(kc):
                vp = ppool.tile([128, 512], f32, name="pp")
                s0 = kc * 128
                for c in range(8):
                    nc.tensor.matmul(
                        vp, X[:, c, s0:s0 + 128], WV[:, c, :],
                        start=(c == 0), stop=(c == 7),
                    )
                nc.vector.tensor_copy(
                    V[:, kc, :, 0:64],
                    vp.rearrange("p (h d) -> p h d", d=64),
                )

            def attn(j, qt, fillers=()):
                # fillers: closures (QKV stripes / V chunks / proj quarters)
                # woven one-per-tt into the PE stream starting at tt=2, so
                # the PE chews prep work while the Act engine exps
                fillers = list(fillers)
                assert len(fillers) <= KC - 2
                qs_ = slice(qt * 512, (qt + 1) * 512)
                outA = otp.tile([128, 512], f32, name="ot")
                outB = otp.tile([128, 512], f32, name="ot")
                for tt in range(KC):
                    sc = scp.tile([128, 1024], f32, name="sc")
                    ks = slice(tt * 128, (tt + 1) * 128)
                    nc.tensor.matmul(
                        sc[:, 0:512], KT[0:64, j, ks], QT[0:64, j, qs_],
                        start=True, stop=True,
                    )
                    nc.tensor.matmul(
                        sc[:, 512:1024], KT[64:128, j, ks], QT[64:128, j, qs_],
                        start=True, stop=True,
                    )
                    at = atp.tile([128, 1024], bf16, name="at")
                    nc.scalar.activation(at, sc, AF.Exp)
                    if tt >= 2 and fillers:
                        fillers.pop(0)()
                    nc.tensor.matmul(
                        outA[0:65, :], V[:, tt, 2 * j, :], at[:, 0:512],
                        start=(tt == 0), stop=(tt == KC - 1),
                    )
                    nc.tensor.matmul(
                        outB[0:65, :], V[:, tt, 2 * j + 1, :], at[:, 512:1024],
                        start=(tt == 0), stop=(tt == KC - 1),
                    )
                for f in fillers:
                    f()
                # normalization is PE-free: DVE copy+recip frees the psum
                # accumulators, Pool broadcasts 1/den across partitions,
                # DVE multiplies into HT
                UA = ivp.tile([65, 512], f32, name="UA")
                UB = ivp.tile([65, 512], f32, name="UB")
                dnA = ivp.tile([1, 512], f32, name="dnA")
                dnB = ivp.tile([1, 512], f32, name="dnB")
                ivA = ivp.tile([1, 512], f32, name="ivA")
                ivB = ivp.tile([1, 512], f32, name="ivB")
                bcA = ivp.tile([64, 512], f32, name="bcA")
                bcB = ivp.tile([64, 512], f32, name="bcB")
                nc.vector.tensor_copy(UA, outA[0:65, :])
                nc.vector.tensor_copy(UB, outB[0:65, :])
                # partition_broadcast only reads partition 0 (and DMA cannot
                # read psum): hop the denominator row down via SBUF->SBUF DMA
                nc.sync.dma_start(dnA, UA[64:65, :])
                nc.sync.dma_start(dnB, UB[64:65, :])
                with nc.allow_low_precision(reason="softmax denom"):
                    nc.vector.reciprocal(ivA, dnA)
                    nc.vector.reciprocal(ivB, dnB)
                nc.gpsimd.partition_broadcast(bcA, ivA)
                nc.gpsimd.partition_broadcast(bcB, ivB)
                # head A lives at HT partitions 0:64 directly
                nc.vector.tensor_mul(HT[0:64, qt, j, :], UA[0:64, :], bcA)
                # head B: compute at base 0, DMA-move to partitions 64:128
                stg = ivp.tile([64, 512], bf16, name="stg")
                nc.vector.tensor_mul(stg, UB[0:64, :], bcB)
                nc.sync.dma_start(HT[64:128, qt, j, :], stg)

            def proj_quarter(qt, q4):
                ob = obp.tile([128, 1024], f32, name="ob")
                rs = slice(q4 * 128, (q4 + 1) * 128)
                for half in range(2):
                    pj = ppool.tile([128, 512], f32, name="pp")
                    for c in range(4):
                        nc.tensor.matmul(
                            pj, HT[:, qt, c, rs],
                            WP[:, c, half * 512:(half + 1) * 512],
                            start=(c == 0), stop=(c == 3),
                        )
                    nc.vector.tensor_copy(
                        ob[:, half * 512:(half + 1) * 512], pj
                    )
                r0 = qt * 512 + q4 * 128
                nc.sync.dma_start(o[r0:r0 + 128, :], ob)

            # ---------- emission schedule ----------
            def F(fn, *a):
                return lambda: fn(*a)

            Q, K, VC, PQ = emit_q_stripe, emit_k_stripe, emit_v_chunk, proj_quarter
            for st in range(ST):
                emit_k_stripe(0, st)
            emit_q_stripe(0, 0)
            emit_q_stripe(0, 1)
            for kc in range(4):
                emit_v_chunk(kc)
            attn(0, 0, [F(VC, kc) for kc in range(4, KC)])
            attn(0, 1, [F(Q, 0, 2), F(Q, 0, 3), F(K, 1, 0)])
            attn(0, 2, [F(K, 1, 1), F(K, 1, 2), F(Q, 1, 0)])
            attn(0, 3, [F(K, 1, 3), F(Q, 1, 1), F(Q, 1, 2)])
            attn(1, 0, [F(Q, 1, 3), F(K, 2, 0)])
            attn(1, 1, [F(K, 2, 1), F(Q, 2, 0)])
            attn(1, 2, [F(K, 2, 2), F(Q, 2, 1)])
            attn(1, 3, [F(K, 2, 3), F(Q, 2, 2)])
            attn(2, 0, [F(Q, 2, 3), F(K, 3, 0)])
            attn(2, 1, [F(K, 3, 1), F(Q, 3, 0)])
            attn(2, 2, [F(K, 3, 2), F(Q, 3, 1)])
            attn(2, 3, [F(K, 3, 3), F(Q, 3, 2)])
            attn(3, 0, [F(Q, 3, 3)])
            attn(3, 1, [F(PQ, 0, q4) for q4 in range(4)])
            attn(3, 2, [F(PQ, 1, q4) for q4 in range(4)])
            attn(3, 3, [F(PQ, 2, q4) for q4 in range(4)])
            for q4 in range(4):
                proj_quarter(3, q4)


def _build_program(s_len=S, repeat=1, timing=False):
    import concourse.bacc as bacc
    import concourse.tile as tile
    from concourse import mybir

    f32 = mybir.dt.float32
    bf16 = mybir.dt.bfloat16
    # timing=True: all real tensors Internal (no host transfer; garbage data
    # has identical cycle counts) + a tiny dummy output, so wall-clock is
    # dispatch + device time only
    kin = "Internal" if timing else "ExternalInput"
    kout = "Internal" if timing else "ExternalOutput"
    nc = bacc.Bacc(
        "TRN2", target_bir_lowering=False, debug=False, num_devices=NCORES
    )
    t = {
        "xT": nc.dram_tensor("xT", [E, s_len], bf16, kind=kin).ap(),
        "wq": nc.dram_tensor("wq", [E, 512], bf16, kind=kin).ap(),
        "wk": nc.dram_tensor("wk", [E, 512], bf16, kind=kin).ap(),
        "wv": nc.dram_tensor("wv", [E, 512], bf16, kind=kin).ap(),
        "wp": nc.dram_tensor("wp", [512, E], bf16, kind=kin).ap(),
        "bq": nc.dram_tensor("bq", [128, 4], f32, kind=kin).ap(),
        "bk": nc.dram_tensor("bk", [128, 4], f32, kind=kin).ap(),
        "ones": nc.dram_tensor("ones", [128, 128], f32, kind=kin).ap(),
        "onesb": nc.dram_tensor(
            "onesb", [128, 128], bf16, kind=kin
        ).ap(),
    }
    o = nc.dram_tensor("o", [s_len, E], f32, kind=kout).ap()
    dummy = (
        nc.dram_tensor("tdum", [1, 4], f32, kind="ExternalOutput").ap()
        if timing else None
    )
    with tile.TileContext(nc) as tc:
        if repeat > 1:
            with tc.For_i(0, repeat, 1):
                _build_body(tc, t, o, s_len)
        else:
            _build_body(tc, t, o, s_len)
        if dummy is not None:
            with tc.tile_pool(name="dum", bufs=1) as dp:
                dt_ = dp.tile([1, 4], f32, name="dum")
                nc.vector.memset(dt_, 0.0)
                nc.sync.dma_start(dummy, dt_)
    nc.compile()
    return nc


def _get_program():
    global _PROGRAM
    if _PROGRAM is None:
        _PROGRAM = _build_program()
    return _PROGRAM


def _shard_inputs(x, w_qkv, b_qkv, w_proj):
    bf16 = ml_dtypes.bfloat16
    wq_f, wk_f, wv_f = w_qkv[:, :E], w_qkv[:, E:2 * E], w_qkv[:, 2 * E:]
    bq_f, bk_f = b_qkv[:E], b_qkv[E:2 * E]
    in_maps = []
    for c in range(NCORES):
        b, hg = divmod(c, 2)
        sl = slice(hg * 512, (hg + 1) * 512)
        in_maps.append({
            "xT": np.ascontiguousarray(x[b].T).astype(bf16),
            "wq": np.ascontiguousarray(wq_f[:, sl]).astype(bf16),
            "wk": np.ascontiguousarray(wk_f[:, sl]).astype(bf16),
            "wv": np.ascontiguousarray(wv_f[:, sl]).astype(bf16),
            "wp": np.ascontiguousarray(w_proj[sl, :]).astype(bf16),
            "bq": np.ascontiguousarray((bq_f[sl] * 0.125).reshape(4, 128).T),
            "bk": np.ascontiguousarray(bk_f[sl].reshape(4, 128).T),
            "ones": np.ones((128, 128), np.float32),
            "onesb": np.ones((128, 128), bf16),
        })
    return in_maps


def kernel(x, w_qkv, b_qkv, w_proj, b_proj):
    global LAST_RESULT
    from concourse.bass_utils import run_bass_kernel_spmd

    x = np.asarray(x, dtype=np.float32)
    w_qkv = np.asarray(w_qkv, dtype=np.float32)
    b_qkv = np.asarray(b_qkv, dtype=np.float32)
    w_proj = np.asarray(w_proj, dtype=np.float32)
    b_proj = np.asarray(b_proj, dtype=np.float32)

    nc = _get_program()
    in_maps = _shard_inputs(x, w_qkv, b_qkv, w_proj)
    res = run_bass_kernel_spmd(nc, in_maps, list(range(NCORES)), trace=TRACE)
    LAST_RESULT = res

    bv_f = b_qkv[2 * E:]
    b_eff = (b_proj + bv_f @ w_proj).astype(np.float32)
    out = np.empty((4, S, E), dtype=np.float32)
    for b in range(4):
        out[b] = res.results[2 * b]["o"] + res.results[2 * b + 1]["o"] + b_eff
    return out


# revision 5
# speedup vs baseline: 1.0897x; 1.0734x over previous
"""Multi-head attention (B=4, S=2048, E=1024, H=16, D=64) on 8 trn2 cores.

Sharding: core c -> (batch b = c//2, head-group hg = c%2 of 8 heads).
Each core computes its 8 heads' attention for its batch plus the partial
output projection (its 512 rows of w_proj); the host sums the two partials
per batch and adds the folded bias (b_proj + b_v @ w_proj).

v2 restructure vs baseline:
  - x is DMA'd ONCE as bf16 [128, 8, 2048] and kept in SBUF (the baseline
    re-streamed x from DRAM five times, leaving phase 1 DMA-bound).
  - all weights are pre-converted to bf16 on host; every matmul runs in
    bf16 (QKV error stays ~1e-3, far under the 2e-2 gate).
  - emission interleaves: QKV(pair 0) -> V -> [attention(pair j) with one
    QKV stripe of pair j+1 woven between q-tiles] -> projection, so the
    scalar engine (exp) gets work early and the PE never drains.
  - normalization reads PV psum once into SBUF (freeing the accumulator
    bank fast) and the broadcast matmuls are deferred into the next
    block's PE stream so the PE doesn't stall on the DVE recip chain.
"""

import ml_dtypes
import numpy as np

S = 2048
E = 1024
NCORES = 8

_PROGRAM = None
TRACE = False
LAST_RESULT = None


def _build_body(tc, t, o, s_len):
    import concourse.bass as bass  # noqa: F401
    from concourse import mybir

    nc = tc.nc
    f32 = mybir.dt.float32
    f32r = mybir.dt.float32r
    bf16 = mybir.dt.bfloat16
    AF = mybir.ActivationFunctionType
    ALU = mybir.AluOpType

    ST = s_len // 512   # number of 512-wide s/q tiles
    KC = s_len // 128   # number of 128-row k chunks

    def r(ap):
        return ap.bitcast(f32r)

    with tc.tile_pool(name="const", bufs=1) as constp, \
         tc.tile_pool(name="big", bufs=1) as bigp:
        X = bigp.tile([128, 8, s_len], bf16, name="X")
        QT = bigp.tile([128, 4, s_len], bf16, name="QT")
        KT = bigp.tile([128, 4, s_len], bf16, name="KT")
        V = bigp.tile([128, KC, 8, 65], bf16, name="V")
        WQ = bigp.tile([128, 8, 512], bf16, name="WQ")
        WK = bigp.tile([128, 8, 512], bf16, name="WK")
        WV = bigp.tile([128, 8, 512], bf16, name="WV")
        WP = bigp.tile([128, 4, 1024], bf16, name="WP")
        HT = bigp.tile([128, ST, 4, 512], bf16, name="HT")
        BQ = constp.tile([128, 4], f32, name="BQ")
        BK = constp.tile([128, 4], f32, name="BK")

        nc.sync.dma_start(BQ, t["bq"])
        nc.sync.dma_start(BK, t["bk"])
        nc.sync.dma_start(WQ, t["wq"].rearrange("(c p) d -> p c d", p=128))
        nc.sync.dma_start(WK, t["wk"].rearrange("(c p) d -> p c d", p=128))
        # ones column for the PV denominator trick: engine-tracked memset
        # (a DMA here has a racy dependency edge against the first PV reads)
        nc.vector.memset(V[:, :, :, 64], 1.0)
        # X ahead of WV/WP: the first K/Q stripes need all of X, while the
        # V matmuls start ~26us in and the projection ~300us in
        xs = t["xT"].rearrange("(c p) s -> p c s", p=128)
        # s-halves first: the first K/Q stripes (st=0,1) only touch
        # s < 1024, so they can start after half the X transfer
        for c in range(8):
            nc.sync.dma_start(X[:, c, 0:1024], xs[:, c, 0:1024])
        for c in range(8):
            nc.sync.dma_start(X[:, c, 1024:2048], xs[:, c, 1024:2048])
        nc.sync.dma_start(WV, t["wv"].rearrange("(c p) d -> p c d", p=128))
        nc.sync.dma_start(WP, t["wp"].rearrange("(c p) e -> p c e", p=128))

        with tc.tile_pool(name="pp", bufs=2, space="PSUM") as ppool, \
             tc.tile_pool(name="at", bufs=6) as atp, \
             tc.tile_pool(name="iv", bufs=2) as ivp, \
             tc.tile_pool(name="ob", bufs=3) as obp, \
             tc.tile_pool(name="sc", bufs=2, space="PSUM") as scp, \
             tc.tile_pool(name="ot", bufs=2, space="PSUM") as otp:

            def emit_q_stripe(j, st):
                ss = slice(st * 512, (st + 1) * 512)
                qp = ppool.tile([128, 512], f32, name="pp")
                for c in range(8):
                    nc.tensor.matmul(
                        qp, WQ[:, c, j * 128:(j + 1) * 128], X[:, c, ss],
                        start=(c == 0), stop=(c == 7),
                    )
                # QT = 0.125 * (x@wq) + 0.125*bq   (bq pre-scaled on host)
                nc.vector.tensor_scalar(
                    QT[:, j, ss], qp, 0.125, BQ[:, j:j + 1], ALU.mult, ALU.add,
                )

            def emit_k_stripe(j, st):
                ss = slice(st * 512, (st + 1) * 512)
                kp = ppool.tile([128, 512], f32, name="pp")
                for c in range(8):
                    nc.tensor.matmul(
                        kp, WK[:, c, j * 128:(j + 1) * 128], X[:, c, ss],
                        start=(c == 0), stop=(c == 7),
                    )
                nc.vector.tensor_scalar(
                    KT[:, j, ss], kp, BK[:, j:j + 1], None, ALU.add,
                )

            def emit_v_chunk(kc):
                vp = ppool.tile([128, 512], f32, name="pp")
                s0 = kc * 128
                for c in range(8):
                    nc.tensor.matmul(
                        vp, X[:, c, s0:s0 + 128], WV[:, c, :],
                        start=(c == 0), stop=(c == 7),
                    )
                nc.vector.tensor_copy(
                    V[:, kc, :, 0:64],
                    vp.rearrange("p (h d) -> p h d", d=64),
                )

            def attn(j, qt, fillers=()):
                # fillers: closures (QKV stripes / V chunks / proj quarters)
                # woven one-per-tt into the PE stream starting at tt=2, so
                # the PE chews prep work while the Act engine exps
                fillers = list(fillers)
                assert len(fillers) <= KC - 2
                qs_ = slice(qt * 512, (qt + 1) * 512)
                outA = otp.tile([128, 512], f32, name="ot")
                outB = otp.tile([128, 512], f32, name="ot")
                for tt in range(KC):
                    sc = scp.tile([128, 1024], f32, name="sc")
                    ks = slice(tt * 128, (tt + 1) * 128)
                    nc.tensor.matmul(
                        sc[:, 0:512], KT[0:64, j, ks], QT[0:64, j, qs_],
                        start=True, stop=True,
                    )
                    nc.tensor.matmul(
                        sc[:, 512:1024], KT[64:128, j, ks], QT[64:128, j, qs_],
                        start=True, stop=True,
                    )
                    at = atp.tile([128, 1024], bf16, name="at")
                    nc.scalar.activation(at, sc, AF.Exp)
                    if tt >= 2 and fillers:
                        fillers.pop(0)()
                    nc.tensor.matmul(
                        outA[0:65, :], V[:, tt, 2 * j, :], at[:, 0:512],
                        start=(tt == 0), stop=(tt == KC - 1),
                    )
                    nc.tensor.matmul(
                        outB[0:65, :], V[:, tt, 2 * j + 1, :], at[:, 512:1024],
                        start=(tt == 0), stop=(tt == KC - 1),
                    )
                for f in fillers:
                    f()
                # normalization is PE-free: DVE copy+recip frees the psum
                # accumulators, Pool broadcasts 1/den across partitions,
                # DVE multiplies into HT
                UA = ivp.tile([65, 512], f32, name="UA")
                UB = ivp.tile([65, 512], f32, name="UB")
                dnA = ivp.tile([1, 512], f32, name="dnA")
                dnB = ivp.tile([1, 512], f32, name="dnB")
                ivA = ivp.tile([1, 512], f32, name="ivA")
                ivB = ivp.tile([1, 512], f32, name="ivB")
                bcA = ivp.tile([64, 512], f32, name="bcA")
                bcB = ivp.tile([64, 512], f32, name="bcB")
                nc.vector.tensor_copy(UA, outA[0:65, :])
                nc.vector.tensor_copy(UB, outB[0:65, :])
                # partition_broadcast only reads partition 0 (and DMA cannot
                # read psum): hop the denominator row down via SBUF->SBUF DMA
                nc.sync.dma_start(dnA, UA[64:65, :])
                nc.sync.dma_start(dnB, UB[64:65, :])
                with nc.allow_low_precision(reason="softmax denom"):
                    nc.vector.reciprocal(ivA, dnA)
                    nc.vector.reciprocal(ivB, dnB)
                nc.gpsimd.partition_broadcast(bcA, ivA)
                nc.gpsimd.partition_broadcast(bcB, ivB)
                # head A lives at HT partitions 0:64 directly
                nc.vector.tensor_mul(HT[0:64, qt, j, :], UA[0:64, :], bcA)
                # head B: compute at base 0, DMA-move to partitions 64:128
                stg = ivp.tile([64, 512], bf16, name="stg")
                nc.vector.tensor_mul(stg, UB[0:64, :], bcB)
                nc.sync.dma_start(HT[64:128, qt, j, :], stg)

            def proj_quarter(qt, q4):
                ob = obp.tile([128, 1024], f32, name="ob")
                rs = slice(q4 * 128, (q4 + 1) * 128)
                for half in range(2):
                    pj = ppool.tile([128, 512], f32, name="pp")
                    for c in range(4):
                        nc.tensor.matmul(
                            pj, HT[:, qt, c, rs],
                            WP[:, c, half * 512:(half + 1) * 512],
                            start=(c == 0), stop=(c == 3),
                        )
                    nc.vector.tensor_copy(
                        ob[:, half * 512:(half + 1) * 512], pj
                    )
                r0 = qt * 512 + q4 * 128
                nc.sync.dma_start(o[r0:r0 + 128, :], ob)

            # ---------- emission schedule ----------
            def F(fn, *a):
                return lambda: fn(*a)

            Q, K, VC, PQ = emit_q_stripe, emit_k_stripe, emit_v_chunk, proj_quarter
            for st in range(ST):
                emit_k_stripe(0, st)
            emit_q_stripe(0, 0)
            emit_q_stripe(0, 1)
            for kc in range(4):
                emit_v_chunk(kc)
            attn(0, 0, [F(VC, kc) for kc in range(4, KC)])
            attn(0, 1, [F(Q, 0, 2), F(Q, 0, 3), F(K, 1, 0)])
            attn(0, 2, [F(K, 1, 1), F(K, 1, 2), F(Q, 1, 0)])
            attn(0, 3, [F(K, 1, 3), F(Q, 1, 1), F(Q, 1, 2)])
            attn(1, 0, [F(Q, 1, 3), F(K, 2, 0)])
            attn(1, 1, [F(K, 2, 1), F(Q, 2, 0)])
            attn(1, 2, [F(K, 2, 2), F(Q, 2, 1)])
            attn(1, 3, [F(K, 2, 3), F(Q, 2, 2)])
            attn(2, 0, [F(Q, 2, 3), F(K, 3, 0)])
            attn(2, 1, [F(K, 3, 1), F(Q, 3, 0)])
            attn(2, 2, [F(K, 3, 2), F(Q, 3, 1)])
            attn(2, 3, [F(K, 3, 3), F(Q, 3, 2)])
            attn(3, 0, [F(Q, 3, 3)])
            attn(3, 1, [F(PQ, 0, q4) for q4 in range(4)])
            attn(3, 2, [F(PQ, 1, q4) for q4 in range(4)])
            attn(3, 3, [F(PQ, 2, q4) for q4 in range(4)])
            for q4 in range(4):
                proj_quarter(3, q4)


def _build_program(s_len=S, repeat=1, timing=False):
    import concourse.bacc as bacc
    import concourse.tile as tile
    from concourse import mybir

    f32 = mybir.dt.float32
    bf16 = mybir.dt.bfloat16
    # timing=True: all real tensors Internal (no host transfer; garbage data
    # has identical cycle counts) + a tiny dummy output, so wall-clock is
    # dispatch + device time only
    kin = "Internal" if timing else "ExternalInput"
    kout = "Internal" if timing else "ExternalOutput"
    nc = bacc.Bacc(
        "TRN2", target_bir_lowering=False, debug=False, num_devices=NCORES
    )
    t = {
        "xT": nc.dram_tensor("xT", [E, s_len], bf16, kind=kin).ap(),
        "wq": nc.dram_tensor("wq", [E, 512], bf16, kind=kin).ap(),
        "wk": nc.dram_tensor("wk", [E, 512], bf16, kind=kin).ap(),
        "wv": nc.dram_tensor("wv", [E, 512], bf16, kind=kin).ap(),
        "wp": nc.dram_tensor("wp", [512, E], bf16, kind=kin).ap(),
        "bq": nc.dram_tensor("bq", [128, 4], f32, kind=kin).ap(),
        "bk": nc.dram_tensor("bk", [128, 4], f32, kind=kin).ap(),
        "ones": nc.dram_tensor("ones", [128, 128], f32, kind=kin).ap(),
        "onesb": nc.dram_tensor(
            "onesb", [128, 128], bf16, kind=kin
        ).ap(),
    }
    o = nc.dram_tensor("o", [s_len, E], f32, kind=kout).ap()
    dummy = (
        nc.dram_tensor("tdum", [1, 4], f32, kind="ExternalOutput").ap()
        if timing else None
    )
    with tile.TileContext(nc) as tc:
        if repeat > 1:
            with tc.For_i(0, repeat, 1):
                _build_body(tc, t, o, s_len)
        else:
            _build_body(tc, t, o, s_len)
        if dummy is not None:
            with tc.tile_pool(name="dum", bufs=1) as dp:
                dt_ = dp.tile([1, 4], f32, name="dum")
                nc.vector.memset(dt_, 0.0)
                nc.sync.dma_start(dummy, dt_)
    nc.compile()
    return nc


def _get_program():
    global _PROGRAM
    if _PROGRAM is None:
        _PROGRAM = _build_program()
    return _PROGRAM


def _shard_inputs(x, w_qkv, b_qkv, w_proj):
    bf16 = ml_dtypes.bfloat16
    wq_f, wk_f, wv_f = w_qkv[:, :E], w_qkv[:, E:2 * E], w_qkv[:, 2 * E:]
    bq_f, bk_f = b_qkv[:E], b_qkv[E:2 * E]
    in_maps = []
    for c in range(NCORES):
        b, hg = divmod(c, 2)
        sl = slice(hg * 512, (hg + 1) * 512)
        in_maps.append({
            "xT": np.ascontiguousarray(x[b].T).astype(bf16),
            "wq": np.ascontiguousarray(wq_f[:, sl]).astype(bf16),
            "wk": np.ascontiguousarray(wk_f[:, sl]).astype(bf16),
            "wv": np.ascontiguousarray(wv_f[:, sl]).astype(bf16),
            "wp": np.ascontiguousarray(w_proj[sl, :]).astype(bf16),
            "bq": np.ascontiguousarray((bq_f[sl] * 0.125).reshape(4, 128).T),
            "bk": np.ascontiguousarray(bk_f[sl].reshape(4, 128).T),
            "ones": np.ones((128, 128), np.float32),
            "onesb": np.ones((128, 128), bf16),
        })
    return in_maps


def kernel(x, w_qkv, b_qkv, w_proj, b_proj):
    global LAST_RESULT
    from concourse.bass_utils import run_bass_kernel_spmd

    x = np.asarray(x, dtype=np.float32)
    w_qkv = np.asarray(w_qkv, dtype=np.float32)
    b_qkv = np.asarray(b_qkv, dtype=np.float32)
    w_proj = np.asarray(w_proj, dtype=np.float32)
    b_proj = np.asarray(b_proj, dtype=np.float32)

    nc = _get_program()
    in_maps = _shard_inputs(x, w_qkv, b_qkv, w_proj)
    res = run_bass_kernel_spmd(nc, in_maps, list(range(NCORES)), trace=TRACE)
    LAST_RESULT = res

    bv_f = b_qkv[2 * E:]
    b_eff = (b_proj + bv_f @ w_proj).astype(np.float32)
    out = np.empty((4, S, E), dtype=np.float32)
    for b in range(4):
        out[b] = res.results[2 * b]["o"] + res.results[2 * b + 1]["o"] + b_eff
    return out
